# revision 33
# baseline (speedup 1.0000x reference)
"""Multi-head attention (B=2, S=2048, nx=768, H=12) on 8 TRN2 NeuronCores.

Sharding: 24 (batch, head) pairs -> 3 heads per core. Core c handles batch
c//4, heads {3*(c%4), +1, +2}. Each core computes QKV projection for its
head slice, attention, and a partial output projection (its 192 rows of
w_proj); the host sums the 4 partials per batch and adds b_proj.

Schedule design (PE-bound kernel; keep the PE back-to-back at full clock):
  - TRN2 PE drops to the 1.2 GHz p-state after ANY idle gap and needs ~3us
    of continuous work to reach 2.4 GHz, so the whole emission order is
    built to never let the PE wait.
  - Scores for head h+1 and PV for head h are interleaved per t-chunk-pair
    with a cycle-metered fill queue (QKV projection blocks, v-projection
    blocks, PV blocks of the previous head, partial output projection).
  - Input DMA is chunked (w_qk by head-chunk, x^T by 512-query slabs) and
    the first two score pairs are woven into qk_proj(0) so the scalar
    engine starts exp'ing ~7us into the kernel.
  - exp granularity [128,1024] f32->bf16 from a 2-tile PSUM ring (4 banks);
    PV accumulates [65,512] per (head, qc) in 4 more banks (ones column in
    V gives softmax denominators for free).
  - Normalization per qc: denominator row -> SBUF, K=1 matmul broadcast to
    64 partitions, DVE reciprocal + multiply (no single-lane reciprocal).
  - Output projection split: heads 0+1 (K=128) projected during stretch C
    into a bf16 partial; head 2 (K=64) + DVE add in the tail, pipelined
    per qc behind head-2 PV sweeps.
"""

import numpy as np
import ml_dtypes

import concourse.bass as bass
import concourse.tile as tile
import concourse.mybir as mybir
from concourse import bacc

BF16 = mybir.dt.bfloat16
F8 = mybir.dt.float8e4
F32 = mybir.dt.float32

NX = 768
D = 64
HPC = 3          # heads per core
N_CORES = 8
KCH = 7          # contraction chunks of 128 (768 data + bias row + pad)
KDIM = KCH * 128  # 896

EXP = mybir.ActivationFunctionType.Exp
EMIT_LOG = []


def build_nc(S=2048):
    TC = S // 128    # key chunks
    QC = S // 512    # query chunks of 512
    nc = bacc.Bacc("TRN2", target_bir_lowering=False, debug=False)

    xt_d = nc.dram_tensor("xt", [KDIM, S], BF16, kind="ExternalInput")
    wqk_d = nc.dram_tensor("wqk", [KDIM, 6 * D], BF16, kind="ExternalInput")
    wv_d = nc.dram_tensor("wv", [KDIM, HPC * D], BF16, kind="ExternalInput")
    wp_d = nc.dram_tensor("wp", [HPC * D, NX], BF16, kind="ExternalInput")
    outa_d = nc.dram_tensor("out_a", [S, NX], BF16, kind="ExternalOutput")
    outbc_d = nc.dram_tensor("out_bc", [S, NX], BF16, kind="ExternalOutput")

    with tile.TileContext(nc) as tc:
        _build_body(tc, outa_d.ap(), outbc_d.ap(), xt_d.ap(), wqk_d.ap(),
                    wv_d.ap(), wp_d.ap(), S, TC, QC)
    nc.compile()
    return nc


def _build_body(tc, outa_d, outbc_d, xt_d, wqk_d, wv_d, wp_d, S, TC, QC):
    nc = tc.nc
    P = 128
    NP = TC // 2     # score pairs per head (8)
    SC = S // 128    # output-projection row chunks (16)

    with tc.tile_pool(name="const", bufs=1) as cpool, \
         tc.tile_pool(name="epool", bufs=40) as epool, \
         tc.tile_pool(name="small", bufs=6) as spool, \
         tc.tile_pool(name="ring", bufs=3, space="PSUM") as ring, \
         tc.tile_pool(name="ps_pv", bufs=2, space="PSUM") as ps_pv:

        # ---- staged inputs ----
        wqk_sb = cpool.tile([P, KCH, 6 * D], BF16)
        wqk_r = wqk_d.rearrange("(c p) m -> p c m", p=P)
        xt_sb = cpool.tile([P, KCH, S], BF16)
        xt_r = xt_d.rearrange("(c p) s -> p c s", p=P)
        wv_sb = cpool.tile([P, KCH, HPC * D], BF16)

        def dma_slab(sl):
            nc.sync.dma_start(xt_sb[:, :, sl * 512:(sl + 1) * 512],
                              xt_r[:, :, sl * 512:(sl + 1) * 512])

        nc.sync.dma_start(wqk_sb[:, :, 0:128], wqk_r[:, :, 0:128])
        dma_slab(0)
        nc.sync.dma_start(wv_sb[:], wv_d.rearrange("(c p) m -> p c m", p=P))
        nc.sync.dma_start(wqk_sb[:, :, 128:384], wqk_r[:, :, 128:384])
        for sl in range(1, QC):
            dma_slab(sl)
        wpa_sb = cpool.tile([D, NX], BF16)
        nc.sync.dma_start(wpa_sb[:], wp_d[0:D, :])
        wpbc_sb = cpool.tile([P, NX], BF16)
        nc.sync.dma_start(wpbc_sb[:], wp_d[D:HPC * D, :])
        ones1 = cpool.tile([1, D], F32)
        nc.vector.memset(ones1[:], 1.0)

        q2_sb = cpool.tile([P, HPC, S], BF16)
        k2_sb = cpool.tile([P, HPC, S // 2], BF16)
        v_sb = cpool.tile([P, TC, HPC, D + 1], BF16)
        nc.vector.memset(v_sb[:, :, :, D:D + 1], 1.0)
        aT_a = cpool.tile([D, S], BF16)    # head 0
        aT_bc = cpool.tile([P, S], BF16)   # heads 1,2 stacked

        # ---- block emitters ------------------------------------------
        def qk_block(mc, qc):
            # one 512-query slab of head mc's [q|k] projection (7 mms)
            EMIT_LOG.extend(["qk"] * KCH)
            ps = ring.tile([P, 1024], F32, tag="ring", name=f"qk_{mc}_{qc}")
            for kc in range(KCH):
                nc.tensor.matmul(
                    ps[:, 0:512],
                    wqk_sb[:, kc, mc * 128:(mc + 1) * 128],
                    xt_sb[:, kc, qc * 512:(qc + 1) * 512],
                    start=(kc == 0), stop=(kc == KCH - 1))
            qsl = slice(qc * 512, (qc + 1) * 512)
            nc.vector.tensor_copy(q2_sb[0:D, mc, qsl], ps[0:D, 0:512])
            nc.vector.tensor_copy(q2_sb[D:P, mc, qsl], q2_sb[0:D, mc, qsl])
            kview = ps[D:P, 0:512].rearrange("p (b c) -> p b c", c=128)
            k2w = k2_sb[:, mc, qc * 256:(qc + 1) * 256].rearrange(
                "p (b c) -> p b c", c=128)
            nc.vector.tensor_copy(k2w[0:D], kview[:, 0::2, :])
            nc.vector.tensor_copy(k2w[D:P], kview[:, 1::2, :])

        def v_block(t):
            EMIT_LOG.extend(["v"] * KCH)
            ps = ring.tile([P, 1024], F32, tag="ring", name=f"v_{t}")
            for kc in range(KCH):
                nc.tensor.matmul(
                    ps[:, 0:HPC * D],
                    xt_sb[:, kc, t * 128:(t + 1) * 128],
                    wv_sb[:, kc, :],
                    start=(kc == 0), stop=(kc == KCH - 1))
            nc.vector.tensor_copy(
                v_sb[:, t, :, 0:D],
                ps[:, 0:HPC * D].rearrange("p (h d) -> p h d", h=HPC))

        e_tiles = {}

        def scores_half(h, j, half, e0, e1):
            # chunk pair (2j, 2j+1), queries half*1024:(half+1)*1024;
            # E is split into [128,1024] L/R tiles per chunk so the first
            # pv sweep (qc 0,1) frees L tiles early (epool pressure).
            EMIT_LOG.extend([f"sc{h}"] * 4)
            for pj, e in ((0, e0), (1, e1)):
                ps = ring.tile([P, 1024], F32, tag="ring",
                               name=f"s{h}_{j}_{half}_{pj}")
                rows = slice(0, D) if pj == 0 else slice(D, P)
                for qq in range(2):
                    qsl = slice((half * 2 + qq) * 512,
                                (half * 2 + qq + 1) * 512)
                    nc.tensor.matmul(
                        ps[:, qq * 512:(qq + 1) * 512],
                        k2_sb[rows, h, j * 128:(j + 1) * 128],
                        q2_sb[rows, h, qsl], start=True, stop=True)
                # exp the whole tile only after both banks are written --
                # an ACT read concurrent with a PE write to the sibling
                # bank of the same tile measurably slows both engines.
                nc.scalar.activation(e[:], ps[:], EXP, scale=0.125)

        def pv_block(h, t, pvs, qc0, last):
            # one qc-pair sweep step: 2 mms; frees this E side when done
            EMIT_LOG.extend([f"pv{h}"] * 2)
            side = "L" if qc0 == 0 else "R"
            e = e_tiles[(h, t, side)]
            for i, qc in enumerate((qc0, qc0 + 1)):
                c = (qc - qc0) * 512
                nc.tensor.matmul(
                    pvs[i][0:D + 1, :],
                    v_sb[:, t, h, :],
                    e[:, c:c + 512],
                    start=(t == 0), stop=(t == TC - 1))
            e_tiles.pop((h, t, side))

        def norm_qc(h, qc, pv_ps):
            # pv_ps: [65, 512] psum (rows 0:64 pv, row 64 sum of exp)
            rt = spool.tile([1, 512], F32, tag="rt", name=f"rt_{h}_{qc}")
            nc.vector.tensor_copy(rt[:], pv_ps[D:D + 1, :])
            EMIT_LOG.append("rb")
            rb = ring.tile([P, 1024], F32, tag="ring", name=f"rb_{h}_{qc}")
            nc.tensor.matmul(rb[0:D, 0:512], ones1[:], rt[:],
                             start=True, stop=True)
            rr = spool.tile([D, 512], F32, tag="rr", name=f"rr_{h}_{qc}")
            nc.vector.reciprocal_approx_fast(rr[:], rb[0:D, 0:512])
            qsl = slice(qc * 512, (qc + 1) * 512)
            dst = (aT_a[:, qsl] if h == 0
                   else aT_bc[(h - 1) * D:h * D, qsl])
            nc.vector.tensor_tensor(dst, pv_ps[0:D, :], rr[:],
                                    mybir.AluOpType.mult)

        def proj_a_block(sc):
            # head-0-only projection (K=64) -> bf16 stage -> DMA
            s_sl = slice(sc * 128, (sc + 1) * 128)
            EMIT_LOG.extend(["pa"] * 2)
            ps = ring.tile([P, 1024], F32, tag="ring", name=f"pa_{sc}")
            for n0, nw in ((0, 512), (512, 256)):
                nc.tensor.matmul(ps[:, n0:n0 + nw], aT_a[:, s_sl],
                                 wpa_sb[:, n0:n0 + nw],
                                 start=True, stop=True)
            ostage = spool.tile([P, NX], BF16, tag="ostage",
                                name=f"oa_{sc}")
            if sc % 2 == 0:
                nc.scalar.copy(ostage[:, 0:512], ps[:, 0:512])
                nc.vector.tensor_copy(ostage[:, 512:NX], ps[:, 512:NX])
            else:
                nc.vector.tensor_copy(ostage[:, 0:512], ps[:, 0:512])
                nc.scalar.copy(ostage[:, 512:NX], ps[:, 512:NX])
            nc.sync.dma_start(outa_d[s_sl, :], ostage[:])

        def proj_bc_block(sc):
            # heads 1+2 projection (K=128) -> bf16 via ACT copy -> DRAM
            s_sl = slice(sc * 128, (sc + 1) * 128)
            EMIT_LOG.extend(["pbc"] * 2)
            ps = ring.tile([P, 1024], F32, tag="ring", name=f"pbc_{sc}")
            for n0, nw in ((0, 512), (512, 256)):
                nc.tensor.matmul(ps[:, n0:n0 + nw], aT_bc[:, s_sl],
                                 wpbc_sb[:, n0:n0 + nw],
                                 start=True, stop=True)
            ostage = spool.tile([P, NX], BF16, tag="ostage",
                                name=f"ostage_{sc}")
            if sc % 2 == 0:
                nc.scalar.copy(ostage[:, 0:512], ps[:, 0:512])
                nc.vector.tensor_copy(ostage[:, 512:NX], ps[:, 512:NX])
                nc.sync.dma_start(outbc_d[s_sl, :], ostage[:])
            else:
                nc.vector.tensor_copy(ostage[:, 0:512], ps[:, 0:512])
                nc.scalar.copy(ostage[:, 512:NX], ps[:, 512:NX])
                # second hardware-DGE queue (ACT) parallelizes the drain
                nc.scalar.dma_start(outbc_d[s_sl, :], ostage[:])

        # ---- fill queue (cycle-metered PE filler) --------------------
        # fill_q holds the CURRENT stretch's eligible items (incl carry
        # from earlier stretches); later-stretch items are staged in
        # next_q and promoted at the stretch boundary, so a stretch can
        # never pull work whose inputs it has not produced yet.
        fill_q = []      # list of (cycles, fn)
        next_q = []
        state = {"i": 0}

        def promote():
            rest = fill_q[state["i"]:]
            fill_q.clear()
            fill_q.extend(rest + next_q)
            next_q.clear()
            state["i"] = 0

        def pull(budget):
            while budget > 0 and state["i"] < len(fill_q):
                c, fn = fill_q[state["i"]]
                state["i"] += 1
                fn()
                budget -= c

        def alloc_pair_e(h, j):
            out = []
            for half, side in ((0, "L"), (1, "R")):
                for c in (2 * j, 2 * j + 1):
                    t = epool.tile([P, 1024], BF16, tag="E",
                                   name=f"e_{h}_{c}_{side}")
                    e_tiles[(h, c, side)] = t
                    out.append(t)
            return out  # [eA_L, eB_L, eA_R, eB_R]

        def scores_pair(h, j, budget_per_half):
            et = alloc_pair_e(h, j)
            for half in range(2):
                scores_half(h, j, half, et[2 * half], et[2 * half + 1])
                pull(budget_per_half)

        # ---- prologue: qk_proj(0) with 2 early score pairs -----------
        # (half0 of a pair only needs q slabs 0,1 and k2 from slab 0)
        qk_block(0, 0)
        qk_block(0, 1)
        pro_e = [alloc_pair_e(0, 0), alloc_pair_e(0, 1)]
        scores_half(0, 0, 0, pro_e[0][0], pro_e[0][1])
        qk_block(0, 2)
        scores_half(0, 1, 0, pro_e[1][0], pro_e[1][1])
        qk_block(0, 3)
        scores_half(0, 0, 1, pro_e[0][2], pro_e[0][3])
        scores_half(0, 1, 1, pro_e[1][2], pro_e[1][3])

        # ---- fill for stretch A: v blocks + qk_proj(1,2) -------------
        for t in range(TC):
            fill_q.append((KCH * 192, (lambda tt: lambda: v_block(tt))(t)))
        for mc in (1, 2):
            for qc in range(QC):
                fill_q.append(
                    (KCH * 512,
                     (lambda m, q: lambda: qk_block(m, q))(mc, qc)))

        PAIR_FILL = 2200   # fill cycles per score half-pair

        # stretch A: scores pairs 2..7 of head 0
        for j in range(2, NP):
            scores_pair(0, j, PAIR_FILL)

        # ---- stretch B: scores(h1) + pv(h0) as fill ------------------
        def add_pv_fill(h, norm_now=True):
            # two qc-pair sweeps; norm each pair right after its sweep
            for sw in range(2):
                pvs = [ps_pv.tile([D + 1, 512], F32, tag="pv",
                                  name=f"pv{h}_{sw}_{i}") for i in range(2)]
                for t in range(TC):
                    fill_q.append(
                        (2 * 512,
                         (lambda tt, pp, q0, la: lambda: pv_block(
                             h, tt, pp, q0, la))(t, pvs, 2 * sw, sw == 1)))
                for i in range(2):
                    # high listed cost forces a pull-break before the next
                    # sweep reuses these psum slots (lets the DVE chain run)
                    fill_q.append(
                        (1800, (lambda q, pp: lambda: norm_qc(h, q, pp))(
                            2 * sw + i, pvs[i])))

        add_pv_fill(0)
        # pre-pull extra fill at the stretch entry: the scalar engine is
        # still draining the previous stretch's exp backlog, so the score
        # psum ring would otherwise block the PE for ~1us per early pair
        pull(4000)
        for j in range(NP):
            scores_pair(1, j, PAIR_FILL)

        # ---- stretch C: scores(h2) + pv(h1) as fill, proj_a ----------
        add_pv_fill(1)
        for sc in range(SC):
            fill_q.append((768, (lambda s: lambda: proj_a_block(s))(sc)))
        pull(4000)
        for j in range(NP):
            scores_pair(2, j, PAIR_FILL)
        pull(10 ** 9)  # drain any remaining fill

        # ---- tail: pv(h2) as two qc-pair sweeps, norm, proj, dma -----
        for sw in range(2):
            side = "L" if sw == 0 else "R"
            pvt = [ps_pv.tile([D + 1, 512], F32, tag="pv",
                              name=f"pv2_{sw}_{i}") for i in range(2)]
            EMIT_LOG.extend(["pv2"] * (2 * TC))
            for t in range(TC):
                e = e_tiles[(2, t, side)]
                for i in range(2):
                    nc.tensor.matmul(
                        pvt[i][0:D + 1, :],
                        v_sb[:, t, 2, :],
                        e[:, i * 512:(i + 1) * 512],
                        start=(t == 0), stop=(t == TC - 1))
            # pbc for the first qc of the pair overlaps the second qc's
            # norm chain (rb -> recip -> mult on DVE)
            norm_qc(2, 2 * sw, pvt[0])
            norm_qc(2, 2 * sw + 1, pvt[1])
            for sc in range(8 * sw, 8 * sw + 4):
                proj_bc_block(sc)
            for sc in range(8 * sw + 4, 8 * sw + 8):
                proj_bc_block(sc)


# ---------------------------------------------------------------------------
# host side
# ---------------------------------------------------------------------------

def make_in_maps(hidden_states, w_attn, b_attn, w_proj, S=2048):
    """Build the 8 per-core input dicts (numpy bf16)."""
    bf = ml_dtypes.bfloat16
    hidden = np.asarray(hidden_states)
    w_attn = np.asarray(w_attn)
    b_attn = np.asarray(b_attn)
    w_proj = np.asarray(w_proj)

    xts = []
    for b in range(hidden.shape[0]):
        xt = np.zeros((KDIM, S), dtype=bf)
        xt[0:NX, :] = hidden[b].T.astype(bf)
        xt[NX, :] = 1.0
        xts.append(xt)

    in_maps = []
    for c in range(N_CORES):
        b = c // (N_CORES // hidden.shape[0])
        h0 = HPC * (c % (N_CORES // hidden.shape[0]))
        wqk = np.zeros((KDIM, 6 * D), dtype=bf)
        wv = np.zeros((KDIM, HPC * D), dtype=bf)
        for i in range(HPC):
            h = h0 + i
            wqk[0:NX, (2 * i) * D:(2 * i + 1) * D] = \
                w_attn[:, h * D:(h + 1) * D].astype(bf)
            wqk[NX, (2 * i) * D:(2 * i + 1) * D] = \
                b_attn[h * D:(h + 1) * D].astype(bf)
            wqk[0:NX, (2 * i + 1) * D:(2 * i + 2) * D] = \
                w_attn[:, NX + h * D:NX + (h + 1) * D].astype(bf)
            wqk[NX, (2 * i + 1) * D:(2 * i + 2) * D] = \
                b_attn[NX + h * D:NX + (h + 1) * D].astype(bf)
            wv[0:NX, i * D:(i + 1) * D] = \
                w_attn[:, 2 * NX + h * D:2 * NX + (h + 1) * D].astype(bf)
            wv[NX, i * D:(i + 1) * D] = \
                b_attn[2 * NX + h * D:2 * NX + (h + 1) * D].astype(bf)
        wp = w_proj[h0 * D:(h0 + HPC) * D, :].astype(bf)
        in_maps.append({"xt": xts[b], "wqk": wqk, "wv": wv, "wp": wp})
    return in_maps


_CACHE = {}


def kernel(hidden_states, w_attn, b_attn, w_proj, b_proj):
    from concourse.bass_utils import run_bass_kernel_spmd

    hidden = np.asarray(hidden_states, dtype=np.float32)
    B, S, _ = hidden.shape
    in_maps = make_in_maps(hidden, w_attn, b_attn, w_proj, S=S)

    if S not in _CACHE:
        _CACHE[S] = build_nc(S=S)
    nc = _CACHE[S]

    res = run_bass_kernel_spmd(nc, in_maps, core_ids=list(range(N_CORES)))
    out = combine_outputs(res.results, B, S)
    out += np.asarray(b_proj, dtype=np.float32)
    return out


def combine_outputs(results, B, S=2048):
    cpb = N_CORES // B
    out = np.zeros((B, S, NX), dtype=np.float32)
    for c in range(N_CORES):
        out[c // cpb] += results[c]["out_a"].astype(np.float32)
        out[c // cpb] += results[c]["out_bc"].astype(np.float32)
    return out
